# revision 1
# baseline (speedup 1.0000x reference)
"""Masked cross-entropy loss (ragged sequences) on 8 Trainium2 NeuronCores.

Problem: loss = sum_{valid} (logsumexp_v(logits[b,s,:]) - logits[b,s,tgt]) / n_valid
where valid = (position k < lengths[b]) & (tgt != 0), logits = output[:, 1:].

Strategy: the only heavy work is the per-token logsumexp over the 32000-wide
vocab (~0.5-1 GB of HBM traffic). The host packs exactly the valid token rows
(k < lengths[b]) into a [T*128, 32000] matrix per core — perfectly
load-balanced sharding over valid tokens — and the device streams each row
once, computing exp on the ScalarE (ACT) engine with per-partition accumulate.
The log(), target-logit gather, masking, and final scalar division are
O(B*S) and stay on the host.

Inputs come as full unsharded arrays; output is the full scalar loss.
"""

import numpy as np

B, SP1, V = 16, 513, 32000
S = SP1 - 1
NCORES = 8
P = 128

_programs = {}        # T (tiles per core) -> compiled Bacc program


def _chunk_schedule(T):
    """Per-tile vocab chunk lists. Tile 0 ramps up so the ACT engine can
    start as soon as a small first DMA lands."""
    ramp = [2000, 2700, 3600, 4800, 6400, 8500, 4000]   # sums to 32000
    assert sum(ramp) == V
    # 16000-wide steady chunks keep the single FIFO HWDGE queue fine-grained
    # enough that prefetches never bubble the ACT engine at tile boundaries
    # (full-tile 32000 chunks measured slower: queue-order stalls).
    return [ramp if j == 0 else [16000, 16000] for j in range(T)]


def _build_program(T):
    """Per-core program: x[T*128, V] bf16 -> se[128, T] f32 where
    se[p, j] = sum_v exp(x[j*128+p, v]). Host applies log()."""
    import concourse.bacc as bacc
    import concourse.tile as tile
    from concourse import mybir

    nc = bacc.Bacc("TRN2", target_bir_lowering=False, debug=False,
                   num_devices=NCORES)
    x = nc.dram_tensor("x", [T * P, V], mybir.dt.bfloat16,
                       kind="ExternalInput").ap()
    se = nc.dram_tensor("se", [P, T], mybir.dt.float32,
                        kind="ExternalOutput").ap()

    sched = _chunk_schedule(T)
    max_nch = max(len(cl) for cl in sched)

    with tile.TileContext(nc) as tc:
        with (
            tc.tile_pool(name="xp", bufs=4) as xp,
            tc.tile_pool(name="scr", bufs=1) as scr,
            tc.tile_pool(name="sm", bufs=2) as sm,
            tc.tile_pool(name="one", bufs=1) as one,
        ):
            total = one.tile([P, T], mybir.dt.float32)
            for j in range(T):
                chunks = sched[j]
                sums = sm.tile([P, max_nch], mybir.dt.float32, tag="sums")
                off = 0
                for c, cw in enumerate(chunks):
                    xt = xp.tile([P, cw], mybir.dt.bfloat16, tag="xt")
                    nc.sync.dma_start(
                        out=xt, in_=x[j * P:(j + 1) * P, off:off + cw])
                    # Scratch holds the (unused) EXP output. ACT's
                    # accumulator yields the per-partition row sum; a DVE
                    # reduce of the chunk would be slower than the EXP
                    # itself and become the bottleneck.
                    et = scr.tile([P, cw], mybir.dt.bfloat16, tag="scr")
                    nc.scalar.activation(
                        et, xt, mybir.ActivationFunctionType.Exp,
                        accum_out=sums[:, c:c + 1])
                    off += cw
                nc.vector.tensor_reduce(
                    out=total[:, j:j + 1], in_=sums[:, :len(chunks)],
                    axis=mybir.AxisListType.X, op=mybir.AluOpType.add)
            nc.sync.dma_start(out=se, in_=total)

    nc.compile()
    return nc


def _get_program(T):
    if T not in _programs:
        _programs[T] = _build_program(T)
    return _programs[T]


def _run_device(in_maps, T, trace=False, tmpdir=None):
    from concourse.bass_utils import run_bass_kernel_spmd

    nc = _get_program(T)
    return run_bass_kernel_spmd(nc, in_maps, core_ids=list(range(NCORES)),
                                trace=trace, tmpdir=tmpdir)


def kernel(output, trg, lengths, _trace=False, _tmpdir=None):
    output = np.asarray(output, dtype=np.float32)
    assert output.shape == (B, SP1, V)
    trg = np.asarray(trg)
    lengths = np.asarray(lengths)

    L = np.clip(lengths.astype(np.int64), 0, S)          # valid tokens per row
    tgt = trg[:, 1:].astype(np.int64)                    # [B, S]

    # Global list of valid tokens (b, k): k < L[b]; logits row = output[b, k+1]
    b_idx = np.repeat(np.arange(B), L)                                  # [N]
    k_idx = np.concatenate([np.arange(n) for n in L]) if L.sum() else \
        np.zeros(0, np.int64)
    n_valid = b_idx.shape[0]
    if n_valid == 0:
        return np.float32(0.0)

    T = -(-n_valid // (NCORES * P))                      # tiles per core
    slots = T * P
    flat = output.reshape(B * SP1, V)
    row_ids = b_idx * SP1 + 1 + k_idx                    # [N] rows in flat
    pad = NCORES * slots - n_valid
    row_ids_p = np.concatenate([row_ids, np.full(pad, row_ids[0])])

    import ml_dtypes

    xin = flat[row_ids_p].astype(ml_dtypes.bfloat16)     # [NCORES*slots, V]
    in_maps = [{"x": xin[m * slots:(m + 1) * slots]} for m in range(NCORES)]
    res = _run_device(in_maps, T, trace=_trace, tmpdir=_tmpdir)

    # se[p, j] on core m -> token m*slots + j*128 + p
    se = np.concatenate(
        [res.results[m]["se"].T.reshape(slots) for m in range(NCORES)]
    )[:n_valid]
    lse = np.log(se.astype(np.float64))

    tgt_tok = tgt[b_idx, k_idx]                          # [N]
    x_tgt = flat[row_ids, tgt_tok]                       # [N] target logits
    keep = tgt_tok != 0                                  # ignore_index=0
    nll = (lse - x_tgt.astype(np.float64)) * keep
    denom = max(float(keep.sum()), 1.0)
    loss = nll.sum() / denom
    out = np.float32(loss)
    if _trace:
        return out, res
    return out

